# revision 29
# baseline (speedup 1.0000x reference)
"""Trainium2 Bass kernel for nn_ByteSequenceEmbedder.

Data-parallel across 8 NeuronCores: 2 sequences per core, weights replicated.

v2 over the 582us baseline:
  * Only the first T_EFF=2176 byte positions are processed (4x512 + 1x128
    column chunks).  max(src_len) for these inputs is 2085; every position
    >= src_len is padding whose conv/highway values never reach a pooled
    word.  If an input batch ever exceeds T_EFF, a full-T=3072 program is
    built lazily (slow compile, still correct).
  * Ragged word max-pool as a single fused DVE scan per channel-chunk:
        state = max(a[t] + state, x2[t]);  a[t] = -1e30 at word starts
    so state at a word's LAST byte is that word's max.  The host gathers
    word-end columns (cumsum-1).  Replaces the 4-op masked-shift max chain
    and halves the mask DMA.
  * Embedding one-hot: the 8 high-vocab rows (256..263) and the bpe-marker
    row are packed into one K=9 matmul (3 matmuls per chunk instead of 4).
  * Output is f16 [seq, 512, T_EFF] (was f32 [seq, 512, 3072]).

Per-core dataflow (activations channels-on-partitions, [C, T] layout):
  embed   : one-hot matmul; tokens broadcast [128,T] (host), DVE is_equal
            vs per-partition iota -> onehot chunks; PE accumulates in PSUM
  conv0   : 3 shifted matmuls per (T-chunk, co-chunk), ReLU+bias in ACT evac
  highway : 2 blocks x 2 layers; 8x4 matmuls per T-chunk, ReLU/Sigmoid evac,
            DVE combine x' = g*(relu(h)-x)+x
  conv1   : 12 matmuls per (T-chunk, co-chunk) + residual add
  pool    : chained tensor_tensor_scan (see above)
  proj    : projection over all T_EFF positions; host selects word-end cols

Matmul operands are bf16 (f32 PSUM accumulation).
"""
import numpy as np

import concourse.bacc as bacc
import concourse.tile as tile
import concourse.mybir as mybir

BSZ, NW, T = 16, 1024, 3072
BED, WED = 128, 512
VOCAB = 264
BPE_MASK_IDX = 4
N_CORES = 8
SEQ_PER_CORE = BSZ // N_CORES
T_EFF = 2176                     # 4*512 + 128; covers max src_len 2085
BF16 = mybir.dt.bfloat16
F16 = mybir.dt.float16
F32 = mybir.dt.float32

_BF16_NP = mybir.dt.np(BF16)
_F16_NP = np.float16
NEG_BIG = -1e30

_CACHE = {}


def _chunks_for(t_eff):
    ch = []
    lo = 0
    while lo < t_eff:
        w = min(512, t_eff - lo)
        ch.append((lo, w))
        lo += w
    return ch


def _build_program(t_eff):
    CHUNKS = _chunks_for(t_eff)
    TE = t_eff
    TP = TE + 2         # conv buffers: one zero halo col each side

    nc = bacc.Bacc("TRN2", target_bir_lowering=False, debug=False)

    def dram_in(name, shape, dt):
        return nc.dram_tensor(name, shape, dt, kind="ExternalInput").ap()

    # embedding lhsT chunks packed into ONE dma: [rows 0..127 | rows 128..255 |
    # (rows 256..263 + bpe row 4) on partitions 0..8]
    emb_pack = dram_in("emb_pack", [128, 3 * 128], BF16)
    # all biases + vocab iota packed into one small f32 dma:
    # iota 0:3 | b_c0 3:7 | b_c1 7:11 | b_hw 11:43 | b_pr 43:47
    misc = dram_in("misc", [128, 47], F32)
    w_c0 = dram_in("w_c0", [128, 3 * WED], BF16)         # [ci, k*512+co]
    w_c1 = dram_in("w_c1", [128, 4 * 3 * WED], BF16)     # [ci%128, (q*3+k)*512+co]
    # highway weights split per (block, layer) so each 1MB piece can be
    # DMA-ordered just ahead of its first use
    w_hws = [dram_in(f"w_hw{bl}", [128, 4 * 1024], BF16) for bl in range(4)]
    w_pr = dram_in("w_pr", [128, 4 * WED], BF16)         # [q*512+co]
    tok_bc = dram_in("tok_bc", [SEQ_PER_CORE, 128, TE], F16)   # tokens bcast
    bpe_row = dram_in("bpe_row", [SEQ_PER_CORE, 1, TE], BF16)  # bpe mask 0/1
    a_msk = dram_in("a_msk", [SEQ_PER_CORE, 128, TE], BF16)    # -1e30 @ word starts

    out = nc.dram_tensor("out", [SEQ_PER_CORE, WED, TE], F16,
                         kind="ExternalOutput").ap()

    RELU = mybir.ActivationFunctionType.Relu
    SIGM = mybir.ActivationFunctionType.Sigmoid
    IDEN = mybir.ActivationFunctionType.Identity
    MAX = mybir.AluOpType.max
    ADD = mybir.AluOpType.add
    SUB = mybir.AluOpType.subtract
    MUL = mybir.AluOpType.mult
    ISEQ = mybir.AluOpType.is_equal

    with tile.TileContext(nc) as tc:
        with tc.tile_pool(name="wp", bufs=1) as wp, \
             tc.tile_pool(name="ap", bufs=1) as apool, \
             tc.tile_pool(name="tp", bufs=3) as tp, \
             tc.tile_pool(name="pp", bufs=8, space="PSUM") as pp:

            # ---- HAM warm-up: PE activity from t~0 so real matmuls start at 2.4GHz ----
            wu = wp.tile([128, 512], BF16)
            nc.vector.memset(wu[:], 0)
            for _ in range(9):
                wps = pp.tile([128, 512], F32, tag="ps", name="wps")
                nc.tensor.matmul(out=wps[:], lhsT=wu[:, 0:128], rhs=wu[:],
                                 start=True, stop=True)

            # ---- load weights/biases once (sync queue, ordered by first use) ----
            t_embP = wp.tile([128, 3 * 128], BF16)
            t_misc = wp.tile([128, 47], F32)
            t_wc0 = wp.tile([128, 3 * WED], BF16)
            t_wc1 = wp.tile([128, 4 * 3 * WED], BF16)
            t_whws = [wp.tile([128, 4 * 1024], BF16, name=f"t_whw{i}")
                      for i in range(4)]
            t_wpr = wp.tile([128, 4 * WED], BF16)
            # first token chunk for seq 0 ahead of the weight loads (critical
            # path to the first real matmul); rest follow on the scalar queue.
            t_tok0 = apool.tile([128, TE], F16, tag="tok", name="t_tok0", bufs=2)
            nc.sync.dma_start(out=t_misc[:], in_=misc[:])
            nc.sync.dma_start(out=t_embP[:], in_=emb_pack[:])
            nc.sync.dma_start(out=t_tok0[:, 0:512], in_=tok_bc[0, :, 0:512])
            for t, d in ((t_wc0, w_c0),
                         (t_whws[0], w_hws[0]), (t_whws[1], w_hws[1]),
                         (t_wc1, w_c1),
                         (t_whws[2], w_hws[2]), (t_whws[3], w_hws[3]),
                         (t_wpr, w_pr)):
                nc.sync.dma_start(out=t[:], in_=d[:])
            t_embA = t_embP[:, 0:256]
            t_embB = t_embP[0:9, 256:384]
            t_iota = t_misc[:, 0:3]
            t_bc0 = t_misc[:, 3:7]
            t_bc1 = t_misc[:, 7:11]
            t_bhw = t_misc[:, 11:43]
            t_bpr = t_misc[:, 43:47]

            def conv0_chunk(X, Y, lo, w):
                """Y[:, chunk m cols 1+lo..] = relu(conv(X) + b) for one chunk."""
                for m in range(4):
                    ps = pp.tile([128, 512], F32, tag="ps", name="ps")
                    for k in range(3):
                        lhs = t_wc0[:, k * WED + m * 128:k * WED + (m + 1) * 128]
                        nc.tensor.matmul(
                            out=ps[:, 0:w], lhsT=lhs,
                            rhs=X[:, lo + k:lo + k + w],
                            start=(k == 0), stop=(k == 2))
                    dst = Y[:, m * TP + 1 + lo:m * TP + 1 + lo + w]
                    nc.scalar.activation(out=dst, in_=ps[:, 0:w], func=RELU,
                                         bias=t_bc0[:, m:m + 1], scale=1.0)

            def highway_chunk(X, Y, bl, lo, w):
                whwt = t_whws[bl]
                pss = []
                for m in range(8):
                    ps = pp.tile([128, 512], F32, tag="ps", name="ps")
                    for q in range(4):
                        base = q * 1024 + m * 128
                        nc.tensor.matmul(
                            out=ps[:, 0:w], lhsT=whwt[:, base:base + 128],
                            rhs=X[:, q * TP + 1 + lo:q * TP + 1 + lo + w],
                            start=(q == 0), stop=(q == 3))
                    pss.append(ps)
                for c in range(4):
                    xs = X[:, c * TP + 1 + lo:c * TP + 1 + lo + w]
                    h_t = tp.tile([128, 512], BF16, tag="h", name="h_t")
                    g_t = tp.tile([128, 512], BF16, tag="g", name="g_t")
                    d_t = tp.tile([128, 512], BF16, tag="d", name="d_t")
                    nc.scalar.activation(out=h_t[:, 0:w], in_=pss[c][:, 0:w], func=RELU,
                                         bias=t_bhw[:, bl * 8 + c:bl * 8 + c + 1], scale=1.0)
                    nc.scalar.activation(out=g_t[:, 0:w], in_=pss[4 + c][:, 0:w], func=SIGM,
                                         bias=t_bhw[:, bl * 8 + 4 + c:bl * 8 + 4 + c + 1], scale=1.0)
                    nc.vector.tensor_tensor(out=d_t[:, 0:w], in0=h_t[:, 0:w], in1=xs, op=SUB)
                    nc.vector.tensor_tensor(out=d_t[:, 0:w], in0=d_t[:, 0:w], in1=g_t[:, 0:w], op=MUL)
                    ys = Y[:, c * TP + 1 + lo:c * TP + 1 + lo + w]
                    nc.vector.tensor_tensor(out=ys, in0=d_t[:, 0:w], in1=xs, op=ADD)

            def highway_layer(X, Y, bl):
                """Y = g*relu(h) + (1-g)*X; X, Y [128, 4*TP]."""
                for (lo, w) in CHUNKS:
                    highway_chunk(X, Y, bl, lo, w)

            scope = nc.named_scope

            def act_buf(tag, bufs=1):
                b = apool.tile([128, 4 * TP], BF16, tag=tag, name=tag, bufs=bufs)
                for q in range(4):
                    nc.vector.memset(b[:, q * TP:q * TP + 1], 0)
                    nc.vector.memset(b[:, q * TP + 1 + TE:(q + 1) * TP], 0)
                return b

            def embed_conv0_seq(s, t_tok=None):
                """Embedding (one-hot matmul) + conv0 for sequence s, chunk
                interleaved (e0 e1 c0 e2 c1 ... c_last) so conv0 work covers
                the per-chunk token DMA latency.  Returns (x1, t_am)."""
                skip0 = t_tok is not None
                ctx = scope(f"s{s}_embed"); ctx.__enter__()
                if t_tok is None:
                    t_tok = apool.tile([128, TE], F16, tag="tok", name="t_tok", bufs=2)
                t_oh3 = apool.tile([9, TE], BF16, tag="oh3b", name="t_oh3", bufs=2)
                t_am = apool.tile([128, TE], BF16, tag="am", name="t_am", bufs=2)
                nc.scalar.dma_start(out=t_oh3[8:9, :], in_=bpe_row[s])
                for ci, (lo, w) in enumerate(CHUNKS):
                    if ci == 0 and skip0:
                        continue
                    nc.scalar.dma_start(out=t_tok[:, lo:lo + w],
                                        in_=tok_bc[s, :, lo:lo + w])
                nc.scalar.dma_start(out=t_am[:], in_=a_msk[s])

                x0 = apool.tile([128, TP], BF16, tag="x0", name="x0", bufs=2)
                nc.vector.memset(x0[:, 0:1], 0)
                nc.vector.memset(x0[:, TP - 1:TP], 0)

                def embed_chunk(lo, w):
                    oh1 = tp.tile([128, 512], BF16, tag="oh1", name="oh1")
                    oh2 = tp.tile([128, 512], BF16, tag="oh2", name="oh2")
                    tb = t_tok[:, lo:lo + w]
                    nc.vector.tensor_scalar(out=oh1[:, 0:w], in0=tb, scalar1=t_iota[:, 0:1],
                                            scalar2=None, op0=ISEQ)
                    nc.vector.tensor_scalar(out=oh2[:, 0:w], in0=tb, scalar1=t_iota[:, 1:2],
                                            scalar2=None, op0=ISEQ)
                    nc.vector.tensor_scalar(out=t_oh3[0:8, lo:lo + w],
                                            in0=t_tok[0:8, lo:lo + w],
                                            scalar1=t_iota[0:8, 2:3], scalar2=None, op0=ISEQ)
                    ps = pp.tile([128, 512], F32, tag="ps", name="ps")
                    nc.tensor.matmul(out=ps[:, 0:w], lhsT=t_embA[:, 0:128], rhs=oh1[:, 0:w],
                                     start=True, stop=False)
                    nc.tensor.matmul(out=ps[:, 0:w], lhsT=t_embA[:, 128:256], rhs=oh2[:, 0:w],
                                     start=False, stop=False)
                    nc.tensor.matmul(out=ps[:, 0:w], lhsT=t_embB[:], rhs=t_oh3[:, lo:lo + w],
                                     start=False, stop=True)
                    nc.scalar.activation(out=x0[:, 1 + lo:1 + lo + w],
                                         in_=ps[:, 0:w], func=IDEN, bias=0.0, scale=1.0)

                x1 = act_buf("actA")
                prev = None
                for i, (lo, w) in enumerate(CHUNKS):
                    embed_chunk(lo, w)
                    if prev is not None:
                        conv0_chunk(x0, x1, *prev)
                    prev = (lo, w)
                conv0_chunk(x0, x1, *prev)
                ctx.__exit__(None, None, None)
                return x1, t_am

            x1, t_am = embed_conv0_seq(0, t_tok=t_tok0)
            for s in range(SEQ_PER_CORE):
                with scope(f"s{s}_hw0l0"):
                    x1b = act_buf("actB", 2)
                    highway_layer(x1, x1b, 0)
                with scope(f"s{s}_hw0l1"):
                    x1c = act_buf("actC")
                    highway_layer(x1b, x1c, 1)

                # ---------- conv1 (+res) + highway block 1 ----------
                ctx = scope(f"s{s}_conv1"); ctx.__enter__()
                x2p = act_buf("actA")
                for (lo, w) in CHUNKS:
                    for m in range(4):
                        ps = pp.tile([128, 512], F32, tag="ps", name="ps")
                        i = 0
                        for q in range(4):
                            for k in range(3):
                                lhs = t_wc1[:, (q * 3 + k) * WED + m * 128:(q * 3 + k) * WED + (m + 1) * 128]
                                nc.tensor.matmul(
                                    out=ps[:, 0:w], lhsT=lhs,
                                    rhs=x1c[:, q * TP + lo + k:q * TP + lo + k + w],
                                    start=(i == 0), stop=(i == 11))
                                i += 1
                        r_t = tp.tile([128, 512], BF16, tag="h", name="r_t")
                        nc.scalar.activation(out=r_t[:, 0:w], in_=ps[:, 0:w], func=RELU,
                                             bias=t_bc1[:, m:m + 1], scale=1.0)
                        xs = x1c[:, m * TP + 1 + lo:m * TP + 1 + lo + w]
                        nc.vector.tensor_tensor(
                            out=x2p[:, m * TP + 1 + lo:m * TP + 1 + lo + w],
                            in0=r_t[:, 0:w], in1=xs, op=ADD)
                ctx.__exit__(None, None, None)

                with scope(f"s{s}_hw1l0"):
                    x2b = act_buf("actB", 2)
                    highway_layer(x2p, x2b, 2)

                # prefetch next sequence's embedding + conv0 BEFORE the fused
                # tail phase: its matmuls slot in ahead so the PE never waits
                # on the scan/evac chains of the final chunks
                if s + 1 < SEQ_PER_CORE:
                    next_x1, next_am = embed_conv0_seq(s + 1)

                # ---------- hw1l1 + ragged max-pool scan + projection, fused ----------
                # Software-pipelined: hw(n); scan(n); proj(n-1) — proj matmuls
                # of the previous chunk hide the evac+combine+scan latency of
                # the current one.
                ctx = scope(f"s{s}_hw1l1pp"); ctx.__enter__()
                x2 = act_buf("actC")
                msel = apool.tile([128, 4 * TE], BF16, tag="msel", name="msel")

                def scan_chunk(lo, w):
                    for c in range(4):
                        init = NEG_BIG if lo == 0 else msel[:, c * TE + lo - 1:c * TE + lo]
                        nc.vector.tensor_tensor_scan(
                            out=msel[:, c * TE + lo:c * TE + lo + w],
                            data0=t_am[:, lo:lo + w],
                            data1=x2[:, c * TP + 1 + lo:c * TP + 1 + lo + w],
                            initial=init, op0=ADD, op1=MAX)

                def proj_chunk(lo, w, last=False):
                    for m in range(4):
                        ps = pp.tile([128, 512], F32, tag="ps", name="ps")
                        for q in range(4):
                            nc.tensor.matmul(
                                out=ps[:, 0:w], lhsT=t_wpr[:, q * WED + m * 128:q * WED + (m + 1) * 128],
                                rhs=msel[:, q * TE + lo:q * TE + lo + w],
                                start=(q == 0), stop=(q == 3))
                        o_t = tp.tile([128, 512], F16, tag="o", name="o_t", bufs=4)
                        nc.scalar.activation(out=o_t[:, 0:w], in_=ps[:, 0:w], func=IDEN,
                                             bias=t_bpr[:, m:m + 1], scale=1.0)
                        nc.sync.dma_start(out=out[s, m * 128:(m + 1) * 128, lo:lo + w],
                                          in_=o_t[:, 0:w])

                prev = None
                for (lo, w) in CHUNKS:
                    highway_chunk(x2b, x2, 3, lo, w)
                    scan_chunk(lo, w)
                    if prev is not None:
                        proj_chunk(*prev)
                    prev = (lo, w)
                proj_chunk(*prev, last=(s == SEQ_PER_CORE - 1))
                ctx.__exit__(None, None, None)
                if s + 1 < SEQ_PER_CORE:
                    x1, t_am = next_x1, next_am

    nc.compile()
    return nc


def _prep_inputs(inputs, t_eff=None):
    """Host-side: shard + convert to the kernel's DRAM tensor layouts."""
    byte_tokens = np.asarray(inputs["byte_tokens"], np.int64)
    bpe_mask = np.asarray(inputs["bpe_mask"], bool)
    pool_lengths = np.asarray(inputs["pool_lengths"], np.int64)
    tok_emb = np.asarray(inputs["tok_emb"], np.float32)
    if t_eff is None:
        t_eff = T_EFF if int(pool_lengths.sum(axis=1).max()) <= T_EFF else T

    def bf(x):
        return np.ascontiguousarray(np.asarray(x, np.float32).astype(_BF16_NP))

    conv0_W = np.asarray(inputs["conv0_W"], np.float32)   # [3,128,512]
    conv1_W = np.asarray(inputs["conv1_W"], np.float32)   # [3,512,512]
    hw0_W = np.asarray(inputs["hw0_W"], np.float32)       # [2,1024,512]
    hw1_W = np.asarray(inputs["hw1_W"], np.float32)
    proj_W = np.asarray(inputs["proj_W"], np.float32)     # [512,512]

    w_c0 = bf(conv0_W.transpose(1, 0, 2).reshape(128, 3 * WED))
    w_c1 = bf(conv1_W.transpose(1, 0, 2).reshape(4, 128, 3, WED)
              .transpose(1, 0, 2, 3).reshape(128, 4 * 3 * WED))
    whw = np.empty((128, 16, 1024), np.float32)
    for bl, (blk, lay) in enumerate(((hw0_W, 0), (hw0_W, 1), (hw1_W, 0), (hw1_W, 1))):
        wt = blk[lay].T  # [512, 1024]
        for q in range(4):
            whw[:, bl * 4 + q, :] = wt[q * 128:(q + 1) * 128]
    w_hw_split = {f"w_hw{bl}": bf(whw[:, bl * 4:(bl + 1) * 4, :].reshape(128, 4096))
                  for bl in range(4)}
    w_pr = bf(proj_W.T.reshape(4, 128, WED).transpose(1, 0, 2).reshape(128, 4 * WED))

    def colchunks(b):  # [512] -> [128, 4]
        return np.ascontiguousarray(np.asarray(b, np.float32).reshape(4, 128).T)

    b_c0 = colchunks(inputs["conv0_b"])
    b_c1 = colchunks(inputs["conv1_b"])
    bhw = np.empty((128, 4, 8), np.float32)
    for bl, (blk, lay) in enumerate((("hw0_b", 0), ("hw0_b", 1), ("hw1_b", 0), ("hw1_b", 1))):
        b = np.asarray(inputs[blk], np.float32)[lay]      # [1024]
        bhw[:, bl, 0:4] = b[:512].reshape(4, 128).T
        bhw[:, bl, 4:8] = b[512:1024].reshape(4, 128).T
    b_hw = np.ascontiguousarray(bhw.reshape(128, 32))
    b_pr = colchunks(inputs["proj_b"])

    # embedding table as packed lhsT row-chunks [128 | 128 | 8+bpe]
    emb_pack = np.zeros((128, 3 * 128), np.float32)
    emb_pack[:, 0:128] = tok_emb[0:128]
    emb_pack[:, 128:256] = tok_emb[128:256]
    emb_pack[0:8, 256:384] = tok_emb[256:264]
    emb_pack[8, 256:384] = tok_emb[BPE_MASK_IDX]
    emb_pack = bf(emb_pack)
    # iota + all biases packed into one small f32 tensor
    misc = np.zeros((128, 47), np.float32)
    p = np.arange(128)
    for j in range(3):
        misc[:, j] = (j * 128 + p).astype(np.float32)
    misc[:, 3:7] = b_c0
    misc[:, 7:11] = b_c1
    misc[:, 11:43] = b_hw
    misc[:, 43:47] = b_pr
    misc = np.ascontiguousarray(misc)

    shared = dict(emb_pack=emb_pack, misc=misc,
                  w_c0=w_c0, w_c1=w_c1, w_pr=w_pr, **w_hw_split)

    in_maps = []
    meta = []
    for core in range(N_CORES):
        m = dict(shared)
        tok = np.empty((SEQ_PER_CORE, 128, t_eff), _F16_NP)
        bpe = np.empty((SEQ_PER_CORE, 1, t_eff), _BF16_NP)
        amsk = np.empty((SEQ_PER_CORE, 128, t_eff), _BF16_NP)
        for s in range(SEQ_PER_CORE):
            b = core * SEQ_PER_CORE + s
            tok[s] = np.broadcast_to(byte_tokens[b, :t_eff].astype(_F16_NP), (128, t_eff))
            bpe[s, 0] = (bpe_mask[b, :t_eff]).astype(_BF16_NP)
            pl = pool_lengths[b]
            cum = np.cumsum(pl)
            s_w = (cum - pl)                      # word starts
            a = np.zeros(t_eff, np.float32)
            a[s_w[s_w < t_eff]] = NEG_BIG
            amsk[s] = np.broadcast_to(a.astype(_BF16_NP), (128, t_eff))
            meta.append((np.clip(cum - 1, 0, t_eff - 1), pl))  # word ends
        m["tok_bc"] = tok
        m["bpe_row"] = bpe
        m["a_msk"] = amsk
        in_maps.append(m)
    return in_maps, meta


def kernel(**inputs) -> np.ndarray:
    from concourse.bass_utils import run_bass_kernel_spmd

    pool_lengths = np.asarray(inputs["pool_lengths"], np.int64)
    t_eff = T_EFF if int(pool_lengths.sum(axis=1).max()) <= T_EFF else T
    key = ("nc", t_eff)
    if key not in _CACHE:
        _CACHE[key] = _build_program(t_eff)
    nc = _CACHE[key]
    _CACHE["nc"] = nc  # convenience handle for test harness profiling

    in_maps, meta = _prep_inputs(inputs, t_eff)
    res = run_bass_kernel_spmd(nc, in_maps, list(range(N_CORES)))

    proj_b = np.asarray(inputs["proj_b"], np.float32)
    full = np.empty((BSZ, NW, WED), np.float32)
    for core in range(N_CORES):
        o = np.asarray(res.results[core]["out"], np.float32)  # [2, 512, t_eff]
        for s in range(SEQ_PER_CORE):
            b = core * SEQ_PER_CORE + s
            e_w, pl = meta[b]
            full[b] = o[s][:, e_w].T
            if (pl == 0).any():
                full[b][pl == 0] = proj_b
    return full


# revision 35
# speedup vs baseline: 1.2006x; 1.2006x over previous
"""Trainium2 Bass kernel for nn_ByteSequenceEmbedder.

Data-parallel across 8 NeuronCores: 2 sequences per core, weights replicated.

v2 over the 582us baseline:
  * Only the first T_EFF=2176 byte positions are processed (4x512 + 1x128
    column chunks).  max(src_len) for these inputs is 2085; every position
    >= src_len is padding whose conv/highway values never reach a pooled
    word.  If an input batch ever exceeds T_EFF, a full-T=3072 program is
    built lazily (slow compile, still correct).
  * Ragged word max-pool as a single fused DVE scan per channel-chunk:
        state = max(a[t] + state, x2[t]);  a[t] = -1e30 at word starts
    so state at a word's LAST byte is that word's max.  The host gathers
    word-end columns (cumsum-1).  Replaces the 4-op masked-shift max chain
    and halves the mask DMA.
  * Embedding one-hot: the 8 high-vocab rows (256..263) and the bpe-marker
    row are packed into one K=9 matmul (3 matmuls per chunk instead of 4).
  * Output is f16 [seq, 512, T_EFF] (was f32 [seq, 512, 3072]).

Per-core dataflow (activations channels-on-partitions, [C, T] layout):
  embed   : one-hot matmul; tokens broadcast [128,T] (host), DVE is_equal
            vs per-partition iota -> onehot chunks; PE accumulates in PSUM
  conv0   : 3 shifted matmuls per (T-chunk, co-chunk), ReLU+bias in ACT evac
  highway : 2 blocks x 2 layers; 8x4 matmuls per T-chunk, ReLU/Sigmoid evac,
            DVE combine x' = g*(relu(h)-x)+x
  conv1   : 12 matmuls per (T-chunk, co-chunk) + residual add
  pool    : chained tensor_tensor_scan (see above)
  proj    : projection over all T_EFF positions; host selects word-end cols

Matmul operands are bf16 (f32 PSUM accumulation).
"""
import numpy as np

import concourse.bacc as bacc
import concourse.tile as tile
import concourse.mybir as mybir

BSZ, NW, T = 16, 1024, 3072
BED, WED = 128, 512
VOCAB = 264
BPE_MASK_IDX = 4
N_CORES = 8
SEQ_PER_CORE = BSZ // N_CORES
T_EFF = 2176                     # 4*512 + 128; covers max src_len 2085
BF16 = mybir.dt.bfloat16
F16 = mybir.dt.float16
F32 = mybir.dt.float32

_BF16_NP = mybir.dt.np(BF16)
_F16_NP = np.float16
NEG_BIG = -1e30

_CACHE = {}


def _chunks_for(t_eff):
    ch = []
    lo = 0
    while lo < t_eff:
        w = min(512, t_eff - lo)
        ch.append((lo, w))
        lo += w
    return ch


def _build_program(t_eff):
    CHUNKS = _chunks_for(t_eff)
    TE = t_eff
    TP = TE + 2         # conv buffers: one zero halo col each side

    nc = bacc.Bacc("TRN2", target_bir_lowering=False, debug=False)

    def dram_in(name, shape, dt):
        return nc.dram_tensor(name, shape, dt, kind="ExternalInput").ap()

    # embedding lhsT chunks packed into ONE dma: [rows 0..127 | rows 128..255 |
    # (rows 256..263 + bpe row 4) on partitions 0..8]
    emb_pack = dram_in("emb_pack", [128, 3 * 128], BF16)
    # all biases + vocab iota packed into one small f32 dma:
    # iota 0:3 | b_c0 3:7 | b_c1 7:11 | b_hw 11:43 | b_pr 43:47
    misc = dram_in("misc", [128, 47], F32)
    w_c0 = dram_in("w_c0", [128, 3 * WED], BF16)         # [ci, k*512+co]
    w_c1 = dram_in("w_c1", [128, 4 * 3 * WED], BF16)     # [ci%128, (q*3+k)*512+co]
    # highway weights split per (block, layer) so each 1MB piece can be
    # DMA-ordered just ahead of its first use
    w_hws = [dram_in(f"w_hw{bl}", [128, 4 * 1024], BF16) for bl in range(4)]
    w_pr = dram_in("w_pr", [128, 4 * WED], BF16)         # [q*512+co]
    tok_bc = dram_in("tok_bc", [SEQ_PER_CORE, 128, TE], F16)   # tokens bcast
    bpe_row = dram_in("bpe_row", [SEQ_PER_CORE, 1, TE], BF16)  # bpe mask 0/1
    a_msk = dram_in("a_msk", [SEQ_PER_CORE, 128, TE], BF16)    # -1e30 @ word starts

    # output channel dim split [4, 128] so one DMA per chunk moves all four
    # partition-chunks (the host reshapes back to [WED, TE])
    out = nc.dram_tensor("out", [SEQ_PER_CORE, 4, 128, TE], F16,
                         kind="ExternalOutput").ap()

    RELU = mybir.ActivationFunctionType.Relu
    SIGM = mybir.ActivationFunctionType.Sigmoid
    IDEN = mybir.ActivationFunctionType.Identity
    MAX = mybir.AluOpType.max
    ADD = mybir.AluOpType.add
    SUB = mybir.AluOpType.subtract
    MUL = mybir.AluOpType.mult
    ISEQ = mybir.AluOpType.is_equal

    with tile.TileContext(nc) as tc:
        with tc.tile_pool(name="wp", bufs=1) as wp, \
             tc.tile_pool(name="ap", bufs=1) as apool, \
             tc.tile_pool(name="tp", bufs=3) as tp, \
             tc.tile_pool(name="pp", bufs=8, space="PSUM") as pp:

            # ---- HAM warm-up: PE activity from t~0 so real matmuls start at 2.4GHz ----
            wu = wp.tile([128, 512], BF16)
            nc.vector.memset(wu[:], 0)
            for _ in range(9):
                wps = pp.tile([128, 512], F32, tag="ps", name="wps")
                nc.tensor.matmul(out=wps[:], lhsT=wu[:, 0:128], rhs=wu[:],
                                 start=True, stop=True)

            # ---- load weights/biases once (sync queue, ordered by first use) ----
            t_embP = wp.tile([128, 3 * 128], BF16)
            t_misc = wp.tile([128, 47], F32)
            t_wc0 = wp.tile([128, 3 * WED], BF16)
            t_wc1 = wp.tile([128, 4 * 3 * WED], BF16)
            t_whws = [wp.tile([128, 4 * 1024], BF16, name=f"t_whw{i}")
                      for i in range(4)]
            t_wpr = wp.tile([128, 4 * WED], BF16)
            # first token chunk for seq 0 ahead of the weight loads (critical
            # path to the first real matmul); rest follow on the scalar queue.
            t_tok0 = apool.tile([128, TE], F16, tag="tok", name="t_tok0", bufs=2)
            nc.sync.dma_start(out=t_misc[:], in_=misc[:])
            nc.sync.dma_start(out=t_embP[:], in_=emb_pack[:])
            nc.sync.dma_start(out=t_tok0[:, 0:512], in_=tok_bc[0, :, 0:512])
            for t, d in ((t_wc0, w_c0),
                         (t_whws[0], w_hws[0]), (t_whws[1], w_hws[1]),
                         (t_wc1, w_c1),
                         (t_whws[2], w_hws[2]), (t_whws[3], w_hws[3]),
                         (t_wpr, w_pr)):
                nc.sync.dma_start(out=t[:], in_=d[:])
            t_embA = t_embP[:, 0:256]
            t_embB = t_embP[0:9, 256:384]
            t_iota = t_misc[:, 0:3]
            t_bc0 = t_misc[:, 3:7]
            t_bc1 = t_misc[:, 7:11]
            t_bhw = t_misc[:, 11:43]
            t_bpr = t_misc[:, 43:47]

            def conv0_chunk(X, Y, lo, w):
                """Y[:, chunk m cols 1+lo..] = relu(conv(X) + b) for one chunk."""
                for m in range(4):
                    ps = pp.tile([128, 512], F32, tag="ps", name="ps")
                    for k in range(3):
                        lhs = t_wc0[:, k * WED + m * 128:k * WED + (m + 1) * 128]
                        nc.tensor.matmul(
                            out=ps[:, 0:w], lhsT=lhs,
                            rhs=X[:, lo + k:lo + k + w],
                            start=(k == 0), stop=(k == 2))
                    dst = Y[:, m * TP + 1 + lo:m * TP + 1 + lo + w]
                    nc.scalar.activation(out=dst, in_=ps[:, 0:w], func=RELU,
                                         bias=t_bc0[:, m:m + 1], scale=1.0)

            def highway_chunk(X, Y, bl, lo, w):
                whwt = t_whws[bl]
                pss = []
                for m in range(8):
                    ps = pp.tile([128, 512], F32, tag="ps", name="ps")
                    for q in range(4):
                        base = q * 1024 + m * 128
                        nc.tensor.matmul(
                            out=ps[:, 0:w], lhsT=whwt[:, base:base + 128],
                            rhs=X[:, q * TP + 1 + lo:q * TP + 1 + lo + w],
                            start=(q == 0), stop=(q == 3))
                    pss.append(ps)
                for c in range(4):
                    xs = X[:, c * TP + 1 + lo:c * TP + 1 + lo + w]
                    h_t = tp.tile([128, 512], BF16, tag="h", name="h_t", bufs=4)
                    g_t = tp.tile([128, 512], BF16, tag="g", name="g_t", bufs=4)
                    d_t = tp.tile([128, 512], BF16, tag="d", name="d_t", bufs=4)
                    nc.scalar.activation(out=h_t[:, 0:w], in_=pss[c][:, 0:w], func=RELU,
                                         bias=t_bhw[:, bl * 8 + c:bl * 8 + c + 1], scale=1.0)
                    nc.scalar.activation(out=g_t[:, 0:w], in_=pss[4 + c][:, 0:w], func=SIGM,
                                         bias=t_bhw[:, bl * 8 + 4 + c:bl * 8 + 4 + c + 1], scale=1.0)
                    nc.vector.tensor_tensor(out=d_t[:, 0:w], in0=h_t[:, 0:w], in1=xs, op=SUB)
                    nc.vector.tensor_tensor(out=d_t[:, 0:w], in0=d_t[:, 0:w], in1=g_t[:, 0:w], op=MUL)
                    ys = Y[:, c * TP + 1 + lo:c * TP + 1 + lo + w]
                    nc.vector.tensor_tensor(out=ys, in0=d_t[:, 0:w], in1=xs, op=ADD)

            def highway_layer(X, Y, bl):
                """Y = g*relu(h) + (1-g)*X; X, Y [128, 4*TP]."""
                for (lo, w) in CHUNKS:
                    highway_chunk(X, Y, bl, lo, w)

            scope = nc.named_scope

            def act_buf(tag, bufs=1):
                b = apool.tile([128, 4 * TP], BF16, tag=tag, name=tag, bufs=bufs)
                for q in range(4):
                    nc.vector.memset(b[:, q * TP:q * TP + 1], 0)
                    nc.vector.memset(b[:, q * TP + 1 + TE:(q + 1) * TP], 0)
                return b

            def embed_conv0_seq(s, t_tok=None):
                """Embedding (one-hot matmul) + conv0 for sequence s, chunk
                interleaved (e0 e1 c0 e2 c1 ... c_last) so conv0 work covers
                the per-chunk token DMA latency.  Returns (x1, t_am)."""
                skip0 = t_tok is not None
                ctx = scope(f"s{s}_embed"); ctx.__enter__()
                if t_tok is None:
                    t_tok = apool.tile([128, TE], F16, tag="tok", name="t_tok", bufs=2)
                t_oh3 = apool.tile([9, TE], BF16, tag="oh3b", name="t_oh3", bufs=2)
                t_am = apool.tile([128, TE], BF16, tag="am", name="t_am", bufs=2)
                nc.scalar.dma_start(out=t_oh3[8:9, :], in_=bpe_row[s])
                for ci, (lo, w) in enumerate(CHUNKS):
                    if ci == 0 and skip0:
                        continue
                    nc.scalar.dma_start(out=t_tok[:, lo:lo + w],
                                        in_=tok_bc[s, :, lo:lo + w])
                nc.scalar.dma_start(out=t_am[:], in_=a_msk[s])

                x0 = apool.tile([128, TP], BF16, tag="x0", name="x0", bufs=2)
                nc.vector.memset(x0[:, 0:1], 0)
                nc.vector.memset(x0[:, TP - 1:TP], 0)

                def embed_chunk(lo, w):
                    oh1 = tp.tile([128, 512], BF16, tag="oh1", name="oh1")
                    oh2 = tp.tile([128, 512], BF16, tag="oh2", name="oh2")
                    tb = t_tok[:, lo:lo + w]
                    nc.vector.tensor_scalar(out=oh1[:, 0:w], in0=tb, scalar1=t_iota[:, 0:1],
                                            scalar2=None, op0=ISEQ)
                    nc.vector.tensor_scalar(out=oh2[:, 0:w], in0=tb, scalar1=t_iota[:, 1:2],
                                            scalar2=None, op0=ISEQ)
                    nc.vector.tensor_scalar(out=t_oh3[0:8, lo:lo + w],
                                            in0=t_tok[0:8, lo:lo + w],
                                            scalar1=t_iota[0:8, 2:3], scalar2=None, op0=ISEQ)
                    ps = pp.tile([128, 512], F32, tag="ps", name="ps")
                    nc.tensor.matmul(out=ps[:, 0:w], lhsT=t_embA[:, 0:128], rhs=oh1[:, 0:w],
                                     start=True, stop=False)
                    nc.tensor.matmul(out=ps[:, 0:w], lhsT=t_embA[:, 128:256], rhs=oh2[:, 0:w],
                                     start=False, stop=False)
                    nc.tensor.matmul(out=ps[:, 0:w], lhsT=t_embB[:], rhs=t_oh3[:, lo:lo + w],
                                     start=False, stop=True)
                    nc.scalar.activation(out=x0[:, 1 + lo:1 + lo + w],
                                         in_=ps[:, 0:w], func=IDEN, bias=0.0, scale=1.0)

                x1 = act_buf("actA")
                prev = None
                for i, (lo, w) in enumerate(CHUNKS):
                    embed_chunk(lo, w)
                    if prev is not None:
                        conv0_chunk(x0, x1, *prev)
                    prev = (lo, w)
                conv0_chunk(x0, x1, *prev)
                ctx.__exit__(None, None, None)
                return x1, t_am

            x1, t_am = embed_conv0_seq(0, t_tok=t_tok0)
            for s in range(SEQ_PER_CORE):
                with scope(f"s{s}_hw0l0"):
                    x1b = act_buf("actB", 2)
                    highway_layer(x1, x1b, 0)
                with scope(f"s{s}_hw0l1"):
                    x1c = act_buf("actC")
                    highway_layer(x1b, x1c, 1)

                # ---------- conv1 (+res) + highway block 1 ----------
                ctx = scope(f"s{s}_conv1"); ctx.__enter__()
                x2p = act_buf("actA")
                for (lo, w) in CHUNKS:
                    for m in range(4):
                        ps = pp.tile([128, 512], F32, tag="ps", name="ps")
                        i = 0
                        for q in range(4):
                            for k in range(3):
                                lhs = t_wc1[:, (q * 3 + k) * WED + m * 128:(q * 3 + k) * WED + (m + 1) * 128]
                                nc.tensor.matmul(
                                    out=ps[:, 0:w], lhsT=lhs,
                                    rhs=x1c[:, q * TP + lo + k:q * TP + lo + k + w],
                                    start=(i == 0), stop=(i == 11))
                                i += 1
                        r_t = tp.tile([128, 512], BF16, tag="h", name="r_t", bufs=4)
                        nc.scalar.activation(out=r_t[:, 0:w], in_=ps[:, 0:w], func=RELU,
                                             bias=t_bc1[:, m:m + 1], scale=1.0)
                        xs = x1c[:, m * TP + 1 + lo:m * TP + 1 + lo + w]
                        nc.vector.tensor_tensor(
                            out=x2p[:, m * TP + 1 + lo:m * TP + 1 + lo + w],
                            in0=r_t[:, 0:w], in1=xs, op=ADD)
                ctx.__exit__(None, None, None)

                with scope(f"s{s}_hw1l0"):
                    x2b = act_buf("actB", 2)
                    highway_layer(x2p, x2b, 2)

                # prefetch next sequence's embedding + conv0 BEFORE the fused
                # tail phase: its matmuls slot in ahead so the PE never waits
                # on the scan/evac chains of the final chunks
                if s + 1 < SEQ_PER_CORE:
                    next_x1, next_am = embed_conv0_seq(s + 1)

                # ---------- hw1l1 + ragged max-pool scan + projection, fused ----------
                # Software-pipelined: hw(n); scan(n); proj(n-1) — proj matmuls
                # of the previous chunk hide the evac+combine+scan latency of
                # the current one.
                ctx = scope(f"s{s}_hw1l1pp"); ctx.__enter__()
                x2 = act_buf("actC")
                msel = apool.tile([128, 4 * TE], BF16, tag="msel", name="msel")

                def scan_chunk(lo, w):
                    for c in range(4):
                        init = NEG_BIG if lo == 0 else msel[:, c * TE + lo - 1:c * TE + lo]
                        nc.vector.tensor_tensor_scan(
                            out=msel[:, c * TE + lo:c * TE + lo + w],
                            data0=t_am[:, lo:lo + w],
                            data1=x2[:, c * TP + 1 + lo:c * TP + 1 + lo + w],
                            initial=init, op0=ADD, op1=MAX)

                def proj_chunk(lo, w, last=False):
                    for m in range(4):
                        ps = pp.tile([128, 512], F32, tag="ps", name="ps")
                        for q in range(4):
                            nc.tensor.matmul(
                                out=ps[:, 0:w], lhsT=t_wpr[:, q * WED + m * 128:q * WED + (m + 1) * 128],
                                rhs=msel[:, q * TE + lo:q * TE + lo + w],
                                start=(q == 0), stop=(q == 3))
                        o_t = tp.tile([128, 512], F16, tag="o", name="o_t", bufs=4)
                        nc.scalar.activation(out=o_t[:, 0:w], in_=ps[:, 0:w],
                                             func=IDEN, bias=t_bpr[:, m:m + 1], scale=1.0)
                        nc.sync.dma_start(out=out[s, m, :, lo:lo + w], in_=o_t[:, 0:w])

                prev = None
                for (lo, w) in CHUNKS:
                    highway_chunk(x2b, x2, 3, lo, w)
                    scan_chunk(lo, w)
                    if prev is not None:
                        proj_chunk(*prev)
                    prev = (lo, w)
                proj_chunk(*prev, last=(s == SEQ_PER_CORE - 1))
                ctx.__exit__(None, None, None)
                if s + 1 < SEQ_PER_CORE:
                    x1, t_am = next_x1, next_am

    nc.compile()
    return nc


def _prep_inputs(inputs, t_eff=None):
    """Host-side: shard + convert to the kernel's DRAM tensor layouts."""
    byte_tokens = np.asarray(inputs["byte_tokens"], np.int64)
    bpe_mask = np.asarray(inputs["bpe_mask"], bool)
    pool_lengths = np.asarray(inputs["pool_lengths"], np.int64)
    tok_emb = np.asarray(inputs["tok_emb"], np.float32)
    if t_eff is None:
        t_eff = T_EFF if int(pool_lengths.sum(axis=1).max()) <= T_EFF else T

    def bf(x):
        return np.ascontiguousarray(np.asarray(x, np.float32).astype(_BF16_NP))

    conv0_W = np.asarray(inputs["conv0_W"], np.float32)   # [3,128,512]
    conv1_W = np.asarray(inputs["conv1_W"], np.float32)   # [3,512,512]
    hw0_W = np.asarray(inputs["hw0_W"], np.float32)       # [2,1024,512]
    hw1_W = np.asarray(inputs["hw1_W"], np.float32)
    proj_W = np.asarray(inputs["proj_W"], np.float32)     # [512,512]

    w_c0 = bf(conv0_W.transpose(1, 0, 2).reshape(128, 3 * WED))
    w_c1 = bf(conv1_W.transpose(1, 0, 2).reshape(4, 128, 3, WED)
              .transpose(1, 0, 2, 3).reshape(128, 4 * 3 * WED))
    whw = np.empty((128, 16, 1024), np.float32)
    for bl, (blk, lay) in enumerate(((hw0_W, 0), (hw0_W, 1), (hw1_W, 0), (hw1_W, 1))):
        wt = blk[lay].T  # [512, 1024]
        for q in range(4):
            whw[:, bl * 4 + q, :] = wt[q * 128:(q + 1) * 128]
    w_hw_split = {f"w_hw{bl}": bf(whw[:, bl * 4:(bl + 1) * 4, :].reshape(128, 4096))
                  for bl in range(4)}
    w_pr = bf(proj_W.T.reshape(4, 128, WED).transpose(1, 0, 2).reshape(128, 4 * WED))

    def colchunks(b):  # [512] -> [128, 4]
        return np.ascontiguousarray(np.asarray(b, np.float32).reshape(4, 128).T)

    b_c0 = colchunks(inputs["conv0_b"])
    b_c1 = colchunks(inputs["conv1_b"])
    bhw = np.empty((128, 4, 8), np.float32)
    for bl, (blk, lay) in enumerate((("hw0_b", 0), ("hw0_b", 1), ("hw1_b", 0), ("hw1_b", 1))):
        b = np.asarray(inputs[blk], np.float32)[lay]      # [1024]
        bhw[:, bl, 0:4] = b[:512].reshape(4, 128).T
        bhw[:, bl, 4:8] = b[512:1024].reshape(4, 128).T
    b_hw = np.ascontiguousarray(bhw.reshape(128, 32))
    b_pr = colchunks(inputs["proj_b"])

    # embedding table as packed lhsT row-chunks [128 | 128 | 8+bpe]
    emb_pack = np.zeros((128, 3 * 128), np.float32)
    emb_pack[:, 0:128] = tok_emb[0:128]
    emb_pack[:, 128:256] = tok_emb[128:256]
    emb_pack[0:8, 256:384] = tok_emb[256:264]
    emb_pack[8, 256:384] = tok_emb[BPE_MASK_IDX]
    emb_pack = bf(emb_pack)
    # iota + all biases packed into one small f32 tensor
    misc = np.zeros((128, 47), np.float32)
    p = np.arange(128)
    for j in range(3):
        misc[:, j] = (j * 128 + p).astype(np.float32)
    misc[:, 3:7] = b_c0
    misc[:, 7:11] = b_c1
    misc[:, 11:43] = b_hw
    misc[:, 43:47] = b_pr
    misc = np.ascontiguousarray(misc)

    shared = dict(emb_pack=emb_pack, misc=misc,
                  w_c0=w_c0, w_c1=w_c1, w_pr=w_pr, **w_hw_split)

    in_maps = []
    meta = []
    for core in range(N_CORES):
        m = dict(shared)
        tok = np.empty((SEQ_PER_CORE, 128, t_eff), _F16_NP)
        bpe = np.empty((SEQ_PER_CORE, 1, t_eff), _BF16_NP)
        amsk = np.empty((SEQ_PER_CORE, 128, t_eff), _BF16_NP)
        for s in range(SEQ_PER_CORE):
            b = core * SEQ_PER_CORE + s
            tok[s] = np.broadcast_to(byte_tokens[b, :t_eff].astype(_F16_NP), (128, t_eff))
            bpe[s, 0] = (bpe_mask[b, :t_eff]).astype(_BF16_NP)
            pl = pool_lengths[b]
            cum = np.cumsum(pl)
            s_w = (cum - pl)                      # word starts
            a = np.zeros(t_eff, np.float32)
            a[s_w[s_w < t_eff]] = NEG_BIG
            amsk[s] = np.broadcast_to(a.astype(_BF16_NP), (128, t_eff))
            meta.append((np.clip(cum - 1, 0, t_eff - 1), pl))  # word ends
        m["tok_bc"] = tok
        m["bpe_row"] = bpe
        m["a_msk"] = amsk
        in_maps.append(m)
    return in_maps, meta


def kernel(**inputs) -> np.ndarray:
    from concourse.bass_utils import run_bass_kernel_spmd

    pool_lengths = np.asarray(inputs["pool_lengths"], np.int64)
    t_eff = T_EFF if int(pool_lengths.sum(axis=1).max()) <= T_EFF else T
    key = ("nc", t_eff)
    if key not in _CACHE:
        _CACHE[key] = _build_program(t_eff)
    nc = _CACHE[key]
    _CACHE["nc"] = nc  # convenience handle for test harness profiling

    in_maps, meta = _prep_inputs(inputs, t_eff)
    res = run_bass_kernel_spmd(nc, in_maps, list(range(N_CORES)))

    proj_b = np.asarray(inputs["proj_b"], np.float32)
    full = np.empty((BSZ, NW, WED), np.float32)
    for core in range(N_CORES):
        o = np.asarray(res.results[core]["out"], np.float32).reshape(
            SEQ_PER_CORE, WED, -1)  # [2, 512, t_eff]
        for s in range(SEQ_PER_CORE):
            b = core * SEQ_PER_CORE + s
            e_w, pl = meta[b]
            full[b] = o[s][:, e_w].T
            if (pl == 0).any():
                full[b][pl == 0] = proj_b
    return full
